# revision 42
# baseline (speedup 1.0000x reference)
"""Trainium2 Bass kernel for nn_DynamicFc (per-sample dynamic MLP).

Strategy: pure data-parallel over 8 NeuronCores (batch 8192 -> 8 x 1024).
Per core, per 128-sample tile:
  f_low   = f @ Wf.T + bf                  (PE, via on-chip transposed f chunks)
  pf_low  = pf @ Wpf.T + bpf               (PE, computed transposed: [low, b])
  params  = pf_low @ pg_w.T (+ pg_b folded via host-side reassociation)
  h = relu(sum_l f_low[b,l] * p1[b,l,m])   (DVE mult+reduce, m-major layout)
  g = sum_m h[b,m] * p2[b,m,l]             (DVE mult+reduce, l-major layout)
  out = g @ W2.T + h @ (B2 @ W2.T) + b2 + f + pf   (PE + DVE/GPSIMD residual)

Bias folding (host-side, exact):
  rhs_f  = [Wf.T | Wf.T @ B1], bias_f = [bf | bf @ B1]  (B1 = pg_b[:4096].reshape(128,32))
  wp     = [W1pT | W2pT]  - pg_w transposed to [low, j] with p1 m-major, p2 l-major
  fin2   = [B2 @ W2.T ; b2]  consumed against [h.T ; ones]
"""

import os
import sys

import numpy as np

for _p in ("/opt/trn_rl_repo", "/root/.axon_site/_ro/trn_rl_repo"):
    if os.path.isdir(_p) and _p not in sys.path:
        sys.path.insert(0, _p)

import bass_rust
import concourse.bass as bass
import concourse.bass_utils as _bu
import concourse.mybir as mybir
import concourse.tile as tile
from concourse.bass_utils import run_bass_kernel_spmd
from concourse.masks import make_identity


def _legalize_waits(nc):
    """walrus (this toolchain) rejects >1 sync wait per instruction; hoist
    extra waits onto preceding NoOps on the same engine."""
    n = 0
    for fn in nc.m.functions:
        for bb in fn.blocks:
            insts = bb.instructions
            out = []
            changed = False
            for inst in insts:
                si = inst.sync_info
                if si is not None and len(si.on_wait) > 1:
                    waits = list(si.on_wait)
                    for w in waits[:-1]:
                        n += 1
                        out.append(
                            mybir.InstNoOp(
                                name=f"I-lw-{n}",
                                engine=inst.engine,
                                sync_info=bass_rust.SyncInfo(on_wait=[w], on_update=[]),
                            )
                        )
                    inst.sync_info = bass_rust.SyncInfo(
                        on_wait=[waits[-1]], on_update=list(si.on_update)
                    )
                    changed = True
                out.append(inst)
            if changed:
                bb.instructions = out
    return n

N_CORES = 8
B, D, LOW, MID = 8192, 1024, 128, 32
DIM = LOW * MID  # 4096
SHARD = B // N_CORES  # 1024
TILE_B = 128
NT = SHARD // TILE_B  # 8
NCHUNK = D // 128  # 8 contraction chunks of the 1024-dim projections

import ml_dtypes

F32 = mybir.dt.float32
CDT = mybir.dt.bfloat16  # compute dtype for matmul operands / prod tiles
NP_CDT = ml_dtypes.bfloat16

_CACHED = {}


def _enable_ldw_opt():
    """Re-enable walrus LDW dedup (safe here: multi-wait legalization already
    keeps instructions single-wait, which was the reason it tripped before)."""
    if _CACHED.get("ldw_patched"):
        return
    orig = _bu.bir_verify_and_optimise

    def patched(tmpdir, inp="bir.json", outp="file.neff", arch=None, *, dve_root=None):
        import pathlib

        cmd = [
            _bu.get_walrus_driver(),
            "--pass",
            "birverifier,runtime_memory_reservation,lower_act,lower_dve,"
            "lower_ap_offset,codegen,neff_packager",
            "-i", inp,
            "--neff-output-filename", outp,
            "--enable-birsim=true", "--mem-mode=physical", "--policy=0",
            "--enable-ldw-opt=true",
            "--assign-static-dmas-to-sp=false",
            "--dram-page-size=256",
            "--enable-neff-debug-info=true",
            "--jobs", "8",
            *_bu.get_walrus_args(
                _bu.get_bir_arch(tmpdir, inp) if arch is None else arch,
                tmpdir, dve_root=dve_root,
            ),
        ]
        r = _bu.run_command(cmd, cwd=tmpdir)
        if r is not None:
            (pathlib.Path(tmpdir) / "log.txt").write_text(r.stdout)
        return f"{tmpdir}/{outp}"

    _bu.bir_verify_and_optimise = patched
    _CACHED["ldw_patched"] = True


def _build_nc():
    nc = bass.Bass()

    f_sh = nc.declare_dram_parameter("f_sh", [SHARD, D], F32, isOutput=False)
    pf_sh = nc.declare_dram_parameter("pf_sh", [SHARD, D], F32, isOutput=False)
    rhs_f = nc.declare_dram_parameter("rhs_f", [D, 160], CDT, isOutput=False)
    bias_f = nc.declare_dram_parameter("bias_f", [1, 160], CDT, isOutput=False)
    rhs_pf = nc.declare_dram_parameter("rhs_pf", [D, LOW], CDT, isOutput=False)
    bias_pf = nc.declare_dram_parameter("bias_pf", [1, LOW], CDT, isOutput=False)
    wp = nc.declare_dram_parameter("wp", [LOW, 2 * DIM], CDT, isOutput=False)
    fin1 = nc.declare_dram_parameter("fin1", [LOW, D], CDT, isOutput=False)
    fin2 = nc.declare_dram_parameter("fin2", [MID + 1, D], CDT, isOutput=False)
    out_sh = nc.declare_dram_parameter("out_sh", [SHARD, D], F32, isOutput=True)

    Relu = mybir.ActivationFunctionType.Relu

    with tile.TileContext(nc) as tc:
        with (
            tc.tile_pool(name="wpool", bufs=1) as wpool,
            tc.tile_pool(name="main", bufs=4) as main,
            tc.tile_pool(name="small", bufs=3) as small,
            tc.tile_pool(name="chunk", bufs=3) as chunk,
            tc.tile_pool(name="pst", bufs=1, space="PSUM") as pst,
            tc.tile_pool(name="pslow", bufs=2, space="PSUM") as pslow,
            tc.tile_pool(name="psbig", bufs=5, space="PSUM") as psbig,
        ):
            # ---- one-time constants / weights ----
            ident = wpool.tile([128, 128], F32)
            make_identity(nc, ident)
            ident_c = wpool.tile([128, 128], CDT)
            make_identity(nc, ident_c)
            ones_row = wpool.tile([1, 128], CDT)
            nc.gpsimd.memset(ones_row, 1.0)

            _loaded0 = None
            tree2_pending = []

            def load_tile(t):
                row = slice(t * TILE_B, (t + 1) * TILE_B)
                r_nat = main.tile([128, D], F32)
                nc.scalar.dma_start(r_nat, r_sh[row, :])
                # transposed loads straight from DRAM via the DMA xbar
                fT = main.tile([128, NCHUNK, 128], CDT)
                nc.sync.dma_start_transpose(fT, f16_sh[row, :])
                pfT = main.tile([128, NCHUNK, 128], CDT)
                nc.sync.dma_start_transpose(pfT, pf16_sh[row, :])
                return row, r_nat, fT, pfT

            def stage_a(t, loaded):
                row, r_nat, fT, pfT = loaded

                # ---- f_low (+h_bias) natural: [b, 160] ----
                ps_fl = pslow.tile([128, 160], F32, tag="lowp")
                for c in range(NCHUNK):
                    nc.tensor.matmul(
                        ps_fl, lhsT=fT[:, c, :], rhs=rhs_f_sb[:, c, :],
                        start=(c == 0), stop=False,
                    )
                nc.tensor.matmul(
                    ps_fl, lhsT=ones_row, rhs=bias_f_sb, start=False, stop=True
                )
                f_low_sb = small.tile([128, LOW], CDT)
                nc.scalar.copy(f_low_sb, ps_fl[:, :LOW])
                h_bias_sb = small.tile([128, MID], F32)
                nc.scalar.copy(h_bias_sb, ps_fl[:, LOW:])

                # ---- pf_low transposed: [low, b] ----
                ps_pl = pslow.tile([128, 128], F32, tag="lowp")
                for c in range(NCHUNK):
                    nc.tensor.matmul(
                        ps_pl, lhsT=rhs_pf_sb[:, c, :], rhs=pfT[:, c, :],
                        start=(c == 0), stop=False,
                    )
                nc.tensor.matmul(
                    ps_pl, lhsT=bias_pf_sb, rhs=ones_row, start=False, stop=True
                )
                pf_lowT_sb = small.tile([128, 128], CDT)
                nc.scalar.copy(pf_lowT_sb, ps_pl)

                # ---- params p1 half (m-major), fused bmm1 ----
                h_dve = small.tile([128, MID], F32)
                prod1 = chunk.tile([128, 32, 128], CDT, tag="prod1", bufs=3)
                for c in range(8):
                    ps_p = psbig.tile([128, 512], F32, tag="big")
                    nc.tensor.matmul(
                        ps_p, lhsT=pf_lowT_sb,
                        rhs=wp_sb[:, c * 512 : (c + 1) * 512],
                        start=True, stop=True,
                    )
                    nc.vector.tensor_mul(
                        prod1[:, c * 4 : (c + 1) * 4, :],
                        ps_p.rearrange("p (m l) -> p m l", l=128),
                        f_low_sb.unsqueeze(1).broadcast_to([128, 4, 128]),
                    )
                w = 64
                while w >= 2:
                    nc.vector.tensor_add(
                        prod1[:, :, 0:w], prod1[:, :, 0:w], prod1[:, :, w : 2 * w]
                    )
                    w //= 2
                nc.vector.tensor_add(
                    h_dve.unsqueeze(2), prod1[:, :, 0:1], prod1[:, :, 1:2]
                )

                # ---- h = relu(h_dve + h_bias) ----
                nc.vector.tensor_add(h_dve, h_dve, h_bias_sb)
                h_sb = small.tile([128, MID], CDT)
                nc.scalar.activation(h_sb, h_dve, Relu)

                # ---- params p2 half (l-major), fused bmm2 ----
                g_bf = small.tile([128, LOW], CDT)
                prod2 = chunk.tile([128, 128, MID], CDT, tag="prod2", bufs=3)
                for c2 in range(4):
                    pchunk = chunk.tile([128, 1024], CDT, tag="pchunk", bufs=6)
                    for hf in range(2):
                        ps_p = psbig.tile([128, 512], F32, tag="big")
                        nc.tensor.matmul(
                            ps_p, lhsT=pf_lowT_sb,
                            rhs=wp_sb[:, DIM + (c2 * 2 + hf) * 512 : DIM + (c2 * 2 + hf + 1) * 512],
                            start=True, stop=True,
                        )
                        nc.scalar.copy(pchunk[:, hf * 512 : (hf + 1) * 512], ps_p)
                    nc.gpsimd.tensor_mul(
                        prod2[:, c2 * 32 : (c2 + 1) * 32, :],
                        pchunk.rearrange("p (l m) -> p l m", m=32),
                        h_sb.unsqueeze(1).broadcast_to([128, 32, 32]),
                    )
                state = (row, r_nat, h_sb, g_bf, prod2)
                tree2_pending.append(state)
                return state

            def finish_tree2(state):
                # bmm2 tree-reduce, placed where GPSIMD(t) has surely drained
                _row, _r, _h, g_bf, prod2 = state
                w = 16
                while w >= 2:
                    nc.vector.tensor_add(
                        prod2[:, :, 0:w], prod2[:, :, 0:w], prod2[:, :, w : 2 * w]
                    )
                    w //= 2
                nc.vector.tensor_add(
                    g_bf.unsqueeze(2), prod2[:, :, 0:1], prod2[:, :, 1:2]
                )

            def stage_b(state):
                row, r_nat, h_sb, g_bf, prod2 = state
                if tree2_pending and tree2_pending[0] is state:
                    finish_tree2(tree2_pending.pop(0))
                # hT_ext = [h.T ; ones]
                ps_ht = pst.tile([MID, 128], CDT, tag="pst")
                nc.tensor.transpose(ps_ht, h_sb, ident_c)
                hT_ext = small.tile([MID + 1, 128], CDT)
                nc.scalar.copy(hT_ext[:MID, :], ps_ht)
                nc.gpsimd.memset(hT_ext[MID : MID + 1, :], 1.0)

                # gT
                ps_gt = pst.tile([128, 128], CDT, tag="pst")
                nc.tensor.transpose(ps_gt, g_bf, ident_c)
                gT_sb = small.tile([128, 128], CDT)
                nc.scalar.copy(gT_sb, ps_gt)

                # ---- final: out = gT.T @ fin1 + hT_ext.T @ fin2 + r ----
                out_sb = main.tile([128, D], F32)
                for hf in range(2):
                    sl = slice(hf * 512, (hf + 1) * 512)
                    ps_o = psbig.tile([128, 512], F32, tag="big")
                    nc.tensor.matmul(
                        ps_o, lhsT=gT_sb, rhs=fin1_sb[:, sl],
                        start=True, stop=False,
                    )
                    nc.tensor.matmul(
                        ps_o, lhsT=hT_ext, rhs=fin2_sb[:, sl],
                        start=False, stop=True,
                    )
                    nc.vector.tensor_add(out_sb[:, sl], ps_o, r_nat[:, sl])
                nc.scalar.dma_start(out_sh[row, :], out_sb)

            rhs_f_sb = wpool.tile([128, NCHUNK, 160], CDT)
            nc.sync.dma_start(rhs_f_sb, rhs_f.rearrange("(c p) n -> p c n", p=128))
            rhs_pf_sb = wpool.tile([128, NCHUNK, LOW], CDT)
            nc.sync.dma_start(rhs_pf_sb, rhs_pf.rearrange("(c p) n -> p c n", p=128))
            bias_f_sb = wpool.tile([1, 160], CDT)
            nc.sync.dma_start(bias_f_sb, bias_f[:, :])
            bias_pf_sb = wpool.tile([1, LOW], CDT)
            nc.sync.dma_start(bias_pf_sb, bias_pf[:, :])
            loads = [load_tile(0), load_tile(1)]
            wp_sb = wpool.tile([LOW, 2 * DIM], CDT)
            nc.sync.dma_start(wp_sb, wp[:, :])
            fin1_sb = wpool.tile([LOW, D], CDT)
            nc.sync.dma_start(fin1_sb, fin1[:, :])
            fin2_sb = wpool.tile([MID + 1, D], CDT)
            nc.sync.dma_start(fin2_sb, fin2[:, :])


            pending = []
            for tp in range(0, NT, 2):
                for t in (tp, tp + 1):
                    if t + 2 < NT:
                        loads.append(load_tile(t + 2))
                    pending.append(stage_a(t, loads.pop(0)))
                if len(pending) > 2:
                    stage_b(pending.pop(0))
                    stage_b(pending.pop(0))
            for st in pending:
                stage_b(st)

    _legalize_waits(nc)
    return nc


def _host_prep(proj_f_w, proj_f_b, proj_pf_w, proj_pf_b, proj_f2_w, proj_f2_b,
               pg_w, pg_b):
    B1 = pg_b[:DIM].reshape(LOW, MID)
    B2 = pg_b[DIM:].reshape(MID, LOW)
    W1pT = pg_w[:DIM].reshape(LOW, MID, LOW).transpose(2, 1, 0).reshape(LOW, DIM)
    W2pT = pg_w[DIM:].reshape(MID, LOW, LOW).transpose(2, 1, 0).reshape(LOW, DIM)
    c = np.ascontiguousarray
    return {
        "rhs_f": c(np.concatenate([proj_f_w.T, proj_f_w.T @ B1], axis=1).astype(NP_CDT)),
        "bias_f": c(np.concatenate([proj_f_b, proj_f_b @ B1])[None, :].astype(NP_CDT)),
        "rhs_pf": c(proj_pf_w.T.astype(NP_CDT)),
        "bias_pf": c(proj_pf_b[None, :].astype(NP_CDT)),
        "wp": c(np.concatenate([W1pT, W2pT], axis=1).astype(NP_CDT)),
        "fin1": c(proj_f2_w.T.astype(NP_CDT)),
        "fin2": c(np.concatenate([B2 @ proj_f2_w.T, proj_f2_b[None, :]], axis=0).astype(NP_CDT)),
    }


def kernel(f, pf, proj_f_w, proj_f_b, proj_pf_w, proj_pf_b, proj_f2_w, proj_f2_b,
           pg_w, pg_b):
    f = np.ascontiguousarray(np.asarray(f, dtype=np.float32))
    pf = np.ascontiguousarray(np.asarray(pf, dtype=np.float32))
    weights = _host_prep(
        np.asarray(proj_f_w, np.float32), np.asarray(proj_f_b, np.float32),
        np.asarray(proj_pf_w, np.float32), np.asarray(proj_pf_b, np.float32),
        np.asarray(proj_f2_w, np.float32), np.asarray(proj_f2_b, np.float32),
        np.asarray(pg_w, np.float32), np.asarray(pg_b, np.float32),
    )

    if "nc" not in _CACHED:
        _CACHED["nc"] = _build_nc()
    nc = _CACHED["nc"]

    in_maps = []
    for i in range(N_CORES):
        m = dict(weights)
        m["f_sh"] = f[i * SHARD : (i + 1) * SHARD]
        m["pf_sh"] = pf[i * SHARD : (i + 1) * SHARD]
        in_maps.append(m)

    res = run_bass_kernel_spmd(nc, in_maps, core_ids=list(range(N_CORES)))
    out = np.concatenate([res.results[i]["out_sh"] for i in range(N_CORES)], axis=0)
    return out


# revision 43
# speedup vs baseline: 1.0287x; 1.0287x over previous
"""Trainium2 Bass kernel for nn_DynamicFc (per-sample dynamic MLP).

Strategy: pure data-parallel over 8 NeuronCores (batch 8192 -> 8 x 1024).
Per core, per 128-sample tile:
  f_low   = f @ Wf.T + bf                  (PE, via on-chip transposed f chunks)
  pf_low  = pf @ Wpf.T + bpf               (PE, computed transposed: [low, b])
  params  = pf_low @ pg_w.T (+ pg_b folded via host-side reassociation)
  h = relu(sum_l f_low[b,l] * p1[b,l,m])   (DVE mult+reduce, m-major layout)
  g = sum_m h[b,m] * p2[b,m,l]             (DVE mult+reduce, l-major layout)
  out = g @ W2.T + h @ (B2 @ W2.T) + b2 + f + pf   (PE + DVE/GPSIMD residual)

Bias folding (host-side, exact):
  rhs_f  = [Wf.T | Wf.T @ B1], bias_f = [bf | bf @ B1]  (B1 = pg_b[:4096].reshape(128,32))
  wp     = [W1pT | W2pT]  - pg_w transposed to [low, j] with p1 m-major, p2 l-major
  fin2   = [B2 @ W2.T ; b2]  consumed against [h.T ; ones]
"""

import os
import sys

import numpy as np

for _p in ("/opt/trn_rl_repo", "/root/.axon_site/_ro/trn_rl_repo"):
    if os.path.isdir(_p) and _p not in sys.path:
        sys.path.insert(0, _p)

import bass_rust
import concourse.bass as bass
import concourse.bass_utils as _bu
import concourse.mybir as mybir
import concourse.tile as tile
from concourse.bass_utils import run_bass_kernel_spmd
from concourse.masks import make_identity


def _legalize_waits(nc):
    """walrus (this toolchain) rejects >1 sync wait per instruction; hoist
    extra waits onto preceding NoOps on the same engine."""
    n = 0
    for fn in nc.m.functions:
        for bb in fn.blocks:
            insts = bb.instructions
            out = []
            changed = False
            for inst in insts:
                si = inst.sync_info
                if si is not None and len(si.on_wait) > 1:
                    waits = list(si.on_wait)
                    for w in waits[:-1]:
                        n += 1
                        out.append(
                            mybir.InstNoOp(
                                name=f"I-lw-{n}",
                                engine=inst.engine,
                                sync_info=bass_rust.SyncInfo(on_wait=[w], on_update=[]),
                            )
                        )
                    inst.sync_info = bass_rust.SyncInfo(
                        on_wait=[waits[-1]], on_update=list(si.on_update)
                    )
                    changed = True
                out.append(inst)
            if changed:
                bb.instructions = out
    return n

N_CORES = 8
B, D, LOW, MID = 8192, 1024, 128, 32
DIM = LOW * MID  # 4096
SHARD = B // N_CORES  # 1024
TILE_B = 128
NT = SHARD // TILE_B  # 8
NCHUNK = D // 128  # 8 contraction chunks of the 1024-dim projections

import ml_dtypes

F32 = mybir.dt.float32
CDT = mybir.dt.bfloat16  # compute dtype for matmul operands / prod tiles
NP_CDT = ml_dtypes.bfloat16

_CACHED = {}


def _enable_ldw_opt():
    """Re-enable walrus LDW dedup (safe here: multi-wait legalization already
    keeps instructions single-wait, which was the reason it tripped before)."""
    if _CACHED.get("ldw_patched"):
        return
    orig = _bu.bir_verify_and_optimise

    def patched(tmpdir, inp="bir.json", outp="file.neff", arch=None, *, dve_root=None):
        import pathlib

        cmd = [
            _bu.get_walrus_driver(),
            "--pass",
            "birverifier,runtime_memory_reservation,lower_act,lower_dve,"
            "lower_ap_offset,codegen,neff_packager",
            "-i", inp,
            "--neff-output-filename", outp,
            "--enable-birsim=true", "--mem-mode=physical", "--policy=0",
            "--enable-ldw-opt=true",
            "--assign-static-dmas-to-sp=false",
            "--dram-page-size=256",
            "--enable-neff-debug-info=true",
            "--jobs", "8",
            *_bu.get_walrus_args(
                _bu.get_bir_arch(tmpdir, inp) if arch is None else arch,
                tmpdir, dve_root=dve_root,
            ),
        ]
        r = _bu.run_command(cmd, cwd=tmpdir)
        if r is not None:
            (pathlib.Path(tmpdir) / "log.txt").write_text(r.stdout)
        return f"{tmpdir}/{outp}"

    _bu.bir_verify_and_optimise = patched
    _CACHED["ldw_patched"] = True


def _build_nc():
    nc = bass.Bass()

    f_sh = nc.declare_dram_parameter("f_sh", [SHARD, D], F32, isOutput=False)
    pf_sh = nc.declare_dram_parameter("pf_sh", [SHARD, D], F32, isOutput=False)
    rhs_f = nc.declare_dram_parameter("rhs_f", [D, 160], CDT, isOutput=False)
    bias_f = nc.declare_dram_parameter("bias_f", [1, 160], CDT, isOutput=False)
    rhs_pf = nc.declare_dram_parameter("rhs_pf", [D, LOW], CDT, isOutput=False)
    bias_pf = nc.declare_dram_parameter("bias_pf", [1, LOW], CDT, isOutput=False)
    wp = nc.declare_dram_parameter("wp", [LOW, 2 * DIM], CDT, isOutput=False)
    fin1 = nc.declare_dram_parameter("fin1", [LOW, D], CDT, isOutput=False)
    fin2 = nc.declare_dram_parameter("fin2", [MID + 1, D], CDT, isOutput=False)
    out_sh = nc.declare_dram_parameter("out_sh", [SHARD, D], F32, isOutput=True)

    Relu = mybir.ActivationFunctionType.Relu

    with tile.TileContext(nc) as tc:
        with (
            tc.tile_pool(name="wpool", bufs=1) as wpool,
            tc.tile_pool(name="main", bufs=4) as main,
            tc.tile_pool(name="small", bufs=3) as small,
            tc.tile_pool(name="chunk", bufs=3) as chunk,
            tc.tile_pool(name="pst", bufs=1, space="PSUM") as pst,
            tc.tile_pool(name="pslow", bufs=2, space="PSUM") as pslow,
            tc.tile_pool(name="psbig", bufs=5, space="PSUM") as psbig,
        ):
            # ---- one-time constants / weights ----
            ident = wpool.tile([128, 128], F32)
            make_identity(nc, ident)
            ident_c = wpool.tile([128, 128], CDT)
            make_identity(nc, ident_c)
            ones_row = wpool.tile([1, 128], CDT)
            nc.gpsimd.memset(ones_row, 1.0)

            _loaded0 = None
            tree2_pending = []

            def load_tile(t):
                row = slice(t * TILE_B, (t + 1) * TILE_B)
                r_nat = main.tile([128, D], F32)
                nc.scalar.dma_start(r_nat, r_sh[row, :])
                # transposed loads straight from DRAM via the DMA xbar
                fT = main.tile([128, NCHUNK, 128], CDT)
                nc.sync.dma_start_transpose(fT, f16_sh[row, :])
                pfT = main.tile([128, NCHUNK, 128], CDT)
                nc.sync.dma_start_transpose(pfT, pf16_sh[row, :])
                return row, r_nat, fT, pfT

            def stage_a(t, loaded):
                row, r_nat, fT, pfT = loaded

                # ---- f_low (+h_bias) natural: [b, 160] ----
                ps_fl = pslow.tile([128, 160], F32, tag="lowp")
                for c in range(NCHUNK):
                    nc.tensor.matmul(
                        ps_fl, lhsT=fT[:, c, :], rhs=rhs_f_sb[:, c, :],
                        start=(c == 0), stop=False,
                    )
                nc.tensor.matmul(
                    ps_fl, lhsT=ones_row, rhs=bias_f_sb, start=False, stop=True
                )
                f_low_sb = small.tile([128, LOW], CDT)
                nc.scalar.copy(f_low_sb, ps_fl[:, :LOW])
                h_bias_sb = small.tile([128, MID], F32)
                nc.scalar.copy(h_bias_sb, ps_fl[:, LOW:])

                # ---- pf_low transposed: [low, b] ----
                ps_pl = pslow.tile([128, 128], F32, tag="lowp")
                for c in range(NCHUNK):
                    nc.tensor.matmul(
                        ps_pl, lhsT=rhs_pf_sb[:, c, :], rhs=pfT[:, c, :],
                        start=(c == 0), stop=False,
                    )
                nc.tensor.matmul(
                    ps_pl, lhsT=bias_pf_sb, rhs=ones_row, start=False, stop=True
                )
                pf_lowT_sb = small.tile([128, 128], CDT)
                nc.scalar.copy(pf_lowT_sb, ps_pl)

                # ---- params p1 half (m-major), fused bmm1 ----
                h_dve = small.tile([128, MID], F32)
                prod1 = chunk.tile([128, 32, 128], CDT, tag="prod1", bufs=3)
                for c in range(8):
                    ps_p = psbig.tile([128, 512], F32, tag="big")
                    nc.tensor.matmul(
                        ps_p, lhsT=pf_lowT_sb,
                        rhs=wp_sb[:, c * 512 : (c + 1) * 512],
                        start=True, stop=True,
                    )
                    nc.vector.tensor_mul(
                        prod1[:, c * 4 : (c + 1) * 4, :],
                        ps_p.rearrange("p (m l) -> p m l", l=128),
                        f_low_sb.unsqueeze(1).broadcast_to([128, 4, 128]),
                    )
                w = 64
                while w >= 2:
                    nc.vector.tensor_add(
                        prod1[:, :, 0:w], prod1[:, :, 0:w], prod1[:, :, w : 2 * w]
                    )
                    w //= 2
                nc.vector.tensor_add(
                    h_dve.unsqueeze(2), prod1[:, :, 0:1], prod1[:, :, 1:2]
                )

                # ---- h = relu(h_dve + h_bias) ----
                nc.vector.tensor_add(h_dve, h_dve, h_bias_sb)
                h_sb = small.tile([128, MID], CDT)
                nc.scalar.activation(h_sb, h_dve, Relu)

                # ---- params p2 half (l-major), fused bmm2 ----
                g_bf = small.tile([128, LOW], CDT)
                prod2 = chunk.tile([128, 128, MID], CDT, tag="prod2", bufs=3)
                for c in range(8):
                    ps_p = psbig.tile([128, 512], F32, tag="big")
                    nc.tensor.matmul(
                        ps_p, lhsT=pf_lowT_sb,
                        rhs=wp_sb[:, DIM + c * 512 : DIM + (c + 1) * 512],
                        start=True, stop=True,
                    )
                    pchunk = chunk.tile([128, 512], CDT, tag="pchunk", bufs=12)
                    nc.scalar.copy(pchunk, ps_p)
                    nc.gpsimd.tensor_mul(
                        prod2[:, c * 16 : (c + 1) * 16, :],
                        pchunk.rearrange("p (l m) -> p l m", m=32),
                        h_sb.unsqueeze(1).broadcast_to([128, 16, 32]),
                    )
                state = (row, r_nat, h_sb, g_bf, prod2)
                tree2_pending.append(state)
                return state

            def finish_tree2(state):
                # bmm2 tree-reduce, placed where GPSIMD(t) has surely drained
                _row, _r, _h, g_bf, prod2 = state
                w = 16
                while w >= 2:
                    nc.vector.tensor_add(
                        prod2[:, :, 0:w], prod2[:, :, 0:w], prod2[:, :, w : 2 * w]
                    )
                    w //= 2
                nc.vector.tensor_add(
                    g_bf.unsqueeze(2), prod2[:, :, 0:1], prod2[:, :, 1:2]
                )

            def stage_b(state):
                row, r_nat, h_sb, g_bf, prod2 = state
                if tree2_pending and tree2_pending[0] is state:
                    finish_tree2(tree2_pending.pop(0))
                # hT_ext = [h.T ; ones]
                ps_ht = pst.tile([MID, 128], CDT, tag="pst")
                nc.tensor.transpose(ps_ht, h_sb, ident_c)
                hT_ext = small.tile([MID + 1, 128], CDT)
                nc.scalar.copy(hT_ext[:MID, :], ps_ht)
                nc.gpsimd.memset(hT_ext[MID : MID + 1, :], 1.0)

                # gT
                ps_gt = pst.tile([128, 128], CDT, tag="pst")
                nc.tensor.transpose(ps_gt, g_bf, ident_c)
                gT_sb = small.tile([128, 128], CDT)
                nc.scalar.copy(gT_sb, ps_gt)

                # ---- final: out = gT.T @ fin1 + hT_ext.T @ fin2 + r ----
                out_sb = main.tile([128, D], F32)
                for hf in range(2):
                    sl = slice(hf * 512, (hf + 1) * 512)
                    ps_o = psbig.tile([128, 512], F32, tag="big")
                    nc.tensor.matmul(
                        ps_o, lhsT=gT_sb, rhs=fin1_sb[:, sl],
                        start=True, stop=False,
                    )
                    nc.tensor.matmul(
                        ps_o, lhsT=hT_ext, rhs=fin2_sb[:, sl],
                        start=False, stop=True,
                    )
                    nc.vector.tensor_add(out_sb[:, sl], ps_o, r_nat[:, sl])
                nc.scalar.dma_start(out_sh[row, :], out_sb)

            rhs_f_sb = wpool.tile([128, NCHUNK, 160], CDT)
            nc.sync.dma_start(rhs_f_sb, rhs_f.rearrange("(c p) n -> p c n", p=128))
            rhs_pf_sb = wpool.tile([128, NCHUNK, LOW], CDT)
            nc.sync.dma_start(rhs_pf_sb, rhs_pf.rearrange("(c p) n -> p c n", p=128))
            bias_f_sb = wpool.tile([1, 160], CDT)
            nc.sync.dma_start(bias_f_sb, bias_f[:, :])
            bias_pf_sb = wpool.tile([1, LOW], CDT)
            nc.sync.dma_start(bias_pf_sb, bias_pf[:, :])
            loads = [load_tile(0), load_tile(1)]
            wp_sb = wpool.tile([LOW, 2 * DIM], CDT)
            nc.sync.dma_start(wp_sb, wp[:, :])
            fin1_sb = wpool.tile([LOW, D], CDT)
            nc.sync.dma_start(fin1_sb, fin1[:, :])
            fin2_sb = wpool.tile([MID + 1, D], CDT)
            nc.sync.dma_start(fin2_sb, fin2[:, :])


            pending = []
            for tp in range(0, NT, 2):
                for t in (tp, tp + 1):
                    if t + 2 < NT:
                        loads.append(load_tile(t + 2))
                    pending.append(stage_a(t, loads.pop(0)))
                if len(pending) > 2:
                    stage_b(pending.pop(0))
                    stage_b(pending.pop(0))
            for st in pending:
                stage_b(st)

    _legalize_waits(nc)
    return nc


def _host_prep(proj_f_w, proj_f_b, proj_pf_w, proj_pf_b, proj_f2_w, proj_f2_b,
               pg_w, pg_b):
    B1 = pg_b[:DIM].reshape(LOW, MID)
    B2 = pg_b[DIM:].reshape(MID, LOW)
    W1pT = pg_w[:DIM].reshape(LOW, MID, LOW).transpose(2, 1, 0).reshape(LOW, DIM)
    W2pT = pg_w[DIM:].reshape(MID, LOW, LOW).transpose(2, 1, 0).reshape(LOW, DIM)
    c = np.ascontiguousarray
    return {
        "rhs_f": c(np.concatenate([proj_f_w.T, proj_f_w.T @ B1], axis=1).astype(NP_CDT)),
        "bias_f": c(np.concatenate([proj_f_b, proj_f_b @ B1])[None, :].astype(NP_CDT)),
        "rhs_pf": c(proj_pf_w.T.astype(NP_CDT)),
        "bias_pf": c(proj_pf_b[None, :].astype(NP_CDT)),
        "wp": c(np.concatenate([W1pT, W2pT], axis=1).astype(NP_CDT)),
        "fin1": c(proj_f2_w.T.astype(NP_CDT)),
        "fin2": c(np.concatenate([B2 @ proj_f2_w.T, proj_f2_b[None, :]], axis=0).astype(NP_CDT)),
    }


def kernel(f, pf, proj_f_w, proj_f_b, proj_pf_w, proj_pf_b, proj_f2_w, proj_f2_b,
           pg_w, pg_b):
    f = np.ascontiguousarray(np.asarray(f, dtype=np.float32))
    pf = np.ascontiguousarray(np.asarray(pf, dtype=np.float32))
    weights = _host_prep(
        np.asarray(proj_f_w, np.float32), np.asarray(proj_f_b, np.float32),
        np.asarray(proj_pf_w, np.float32), np.asarray(proj_pf_b, np.float32),
        np.asarray(proj_f2_w, np.float32), np.asarray(proj_f2_b, np.float32),
        np.asarray(pg_w, np.float32), np.asarray(pg_b, np.float32),
    )

    if "nc" not in _CACHED:
        _CACHED["nc"] = _build_nc()
    nc = _CACHED["nc"]

    in_maps = []
    for i in range(N_CORES):
        m = dict(weights)
        m["f_sh"] = f[i * SHARD : (i + 1) * SHARD]
        m["pf_sh"] = pf[i * SHARD : (i + 1) * SHARD]
        in_maps.append(m)

    res = run_bass_kernel_spmd(nc, in_maps, core_ids=list(range(N_CORES)))
    out = np.concatenate([res.results[i]["out_sh"] for i in range(N_CORES)], axis=0)
    return out


# revision 44
# speedup vs baseline: 1.0309x; 1.0022x over previous
"""Trainium2 Bass kernel for nn_DynamicFc (per-sample dynamic MLP).

Strategy: pure data-parallel over 8 NeuronCores (batch 8192 -> 8 x 1024).
Per core, per 128-sample tile:
  f_low   = f @ Wf.T + bf                  (PE, via on-chip transposed f chunks)
  pf_low  = pf @ Wpf.T + bpf               (PE, computed transposed: [low, b])
  params  = pf_low @ pg_w.T (+ pg_b folded via host-side reassociation)
  h = relu(sum_l f_low[b,l] * p1[b,l,m])   (DVE mult+reduce, m-major layout)
  g = sum_m h[b,m] * p2[b,m,l]             (DVE mult+reduce, l-major layout)
  out = g @ W2.T + h @ (B2 @ W2.T) + b2 + f + pf   (PE + DVE/GPSIMD residual)

Bias folding (host-side, exact):
  rhs_f  = [Wf.T | Wf.T @ B1], bias_f = [bf | bf @ B1]  (B1 = pg_b[:4096].reshape(128,32))
  wp     = [W1pT | W2pT]  - pg_w transposed to [low, j] with p1 m-major, p2 l-major
  fin2   = [B2 @ W2.T ; b2]  consumed against [h.T ; ones]
"""

import os
import sys

import numpy as np

for _p in ("/opt/trn_rl_repo", "/root/.axon_site/_ro/trn_rl_repo"):
    if os.path.isdir(_p) and _p not in sys.path:
        sys.path.insert(0, _p)

import bass_rust
import concourse.bass as bass
import concourse.bass_utils as _bu
import concourse.mybir as mybir
import concourse.tile as tile
from concourse.bass_utils import run_bass_kernel_spmd
from concourse.masks import make_identity


def _legalize_waits(nc):
    """walrus (this toolchain) rejects >1 sync wait per instruction; hoist
    extra waits onto preceding NoOps on the same engine."""
    n = 0
    for fn in nc.m.functions:
        for bb in fn.blocks:
            insts = bb.instructions
            out = []
            changed = False
            for inst in insts:
                si = inst.sync_info
                if si is not None and len(si.on_wait) > 1:
                    waits = list(si.on_wait)
                    for w in waits[:-1]:
                        n += 1
                        out.append(
                            mybir.InstNoOp(
                                name=f"I-lw-{n}",
                                engine=inst.engine,
                                sync_info=bass_rust.SyncInfo(on_wait=[w], on_update=[]),
                            )
                        )
                    inst.sync_info = bass_rust.SyncInfo(
                        on_wait=[waits[-1]], on_update=list(si.on_update)
                    )
                    changed = True
                out.append(inst)
            if changed:
                bb.instructions = out
    return n

N_CORES = 8
B, D, LOW, MID = 8192, 1024, 128, 32
DIM = LOW * MID  # 4096
SHARD = B // N_CORES  # 1024
TILE_B = 128
NT = SHARD // TILE_B  # 8
NCHUNK = D // 128  # 8 contraction chunks of the 1024-dim projections

import ml_dtypes

F32 = mybir.dt.float32
CDT = mybir.dt.bfloat16  # compute dtype for matmul operands / prod tiles
NP_CDT = ml_dtypes.bfloat16

_CACHED = {}


def _enable_ldw_opt():
    """Re-enable walrus LDW dedup (safe here: multi-wait legalization already
    keeps instructions single-wait, which was the reason it tripped before)."""
    if _CACHED.get("ldw_patched"):
        return
    orig = _bu.bir_verify_and_optimise

    def patched(tmpdir, inp="bir.json", outp="file.neff", arch=None, *, dve_root=None):
        import pathlib

        cmd = [
            _bu.get_walrus_driver(),
            "--pass",
            "birverifier,runtime_memory_reservation,lower_act,lower_dve,"
            "lower_ap_offset,codegen,neff_packager",
            "-i", inp,
            "--neff-output-filename", outp,
            "--enable-birsim=true", "--mem-mode=physical", "--policy=0",
            "--enable-ldw-opt=true",
            "--assign-static-dmas-to-sp=false",
            "--dram-page-size=256",
            "--enable-neff-debug-info=true",
            "--jobs", "8",
            *_bu.get_walrus_args(
                _bu.get_bir_arch(tmpdir, inp) if arch is None else arch,
                tmpdir, dve_root=dve_root,
            ),
        ]
        r = _bu.run_command(cmd, cwd=tmpdir)
        if r is not None:
            (pathlib.Path(tmpdir) / "log.txt").write_text(r.stdout)
        return f"{tmpdir}/{outp}"

    _bu.bir_verify_and_optimise = patched
    _CACHED["ldw_patched"] = True


def _build_nc():
    nc = bass.Bass()

    f_sh = nc.declare_dram_parameter("f_sh", [SHARD, D], F32, isOutput=False)
    pf_sh = nc.declare_dram_parameter("pf_sh", [SHARD, D], F32, isOutput=False)
    rhs_f = nc.declare_dram_parameter("rhs_f", [D, 160], CDT, isOutput=False)
    bias_f = nc.declare_dram_parameter("bias_f", [1, 160], CDT, isOutput=False)
    rhs_pf = nc.declare_dram_parameter("rhs_pf", [D, LOW], CDT, isOutput=False)
    bias_pf = nc.declare_dram_parameter("bias_pf", [1, LOW], CDT, isOutput=False)
    wp = nc.declare_dram_parameter("wp", [LOW, 2 * DIM], CDT, isOutput=False)
    fin1 = nc.declare_dram_parameter("fin1", [LOW, D], CDT, isOutput=False)
    fin2 = nc.declare_dram_parameter("fin2", [MID + 1, D], CDT, isOutput=False)
    out_sh = nc.declare_dram_parameter("out_sh", [SHARD, D], F32, isOutput=True)

    Relu = mybir.ActivationFunctionType.Relu

    with tile.TileContext(nc) as tc:
        with (
            tc.tile_pool(name="wpool", bufs=1) as wpool,
            tc.tile_pool(name="main", bufs=4) as main,
            tc.tile_pool(name="small", bufs=3) as small,
            tc.tile_pool(name="chunk", bufs=3) as chunk,
            tc.tile_pool(name="pst", bufs=1, space="PSUM") as pst,
            tc.tile_pool(name="pslow", bufs=2, space="PSUM") as pslow,
            tc.tile_pool(name="psbig", bufs=5, space="PSUM") as psbig,
        ):
            # ---- one-time constants / weights ----
            ident = wpool.tile([128, 128], F32)
            make_identity(nc, ident)
            ident_c = wpool.tile([128, 128], CDT)
            make_identity(nc, ident_c)
            ones_row = wpool.tile([1, 128], CDT)
            nc.gpsimd.memset(ones_row, 1.0)

            _loaded0 = None
            tree2_pending = []

            def load_tile(t):
                row = slice(t * TILE_B, (t + 1) * TILE_B)
                r_nat = main.tile([128, D], F32)
                nc.scalar.dma_start(r_nat, r_sh[row, :])
                # transposed loads straight from DRAM via the DMA xbar
                fT = main.tile([128, NCHUNK, 128], CDT)
                nc.sync.dma_start_transpose(fT, f16_sh[row, :])
                pfT = main.tile([128, NCHUNK, 128], CDT)
                nc.sync.dma_start_transpose(pfT, pf16_sh[row, :])
                return row, r_nat, fT, pfT

            def make_low(loaded):
                # low-phase matmuls as step closures, interleaved into the
                # previous tile's params phase as in-order-PE filler work
                row, r_nat, fT, pfT = loaded
                ps_fl = pslow.tile([128, 160], F32, tag="lowp")
                ps_pl = pslow.tile([128, 128], F32, tag="lowp")
                f_low_sb = small.tile([128, LOW], CDT)
                h_bias_sb = small.tile([128, MID], F32)
                pf_lowT_sb = small.tile([128, 128], CDT)

                def fl_mm(c):
                    nc.tensor.matmul(
                        ps_fl, lhsT=fT[:, c, :], rhs=rhs_f_sb[:, c, :],
                        start=(c == 0), stop=False,
                    )

                def fl_done():
                    nc.tensor.matmul(
                        ps_fl, lhsT=ones_row, rhs=bias_f_sb, start=False, stop=True
                    )
                    nc.scalar.copy(f_low_sb, ps_fl[:, :LOW])
                    nc.scalar.copy(h_bias_sb, ps_fl[:, LOW:])

                def pl_mm(c):
                    nc.tensor.matmul(
                        ps_pl, lhsT=rhs_pf_sb[:, c, :], rhs=pfT[:, c, :],
                        start=(c == 0), stop=False,
                    )

                def pl_done():
                    nc.tensor.matmul(
                        ps_pl, lhsT=bias_pf_sb, rhs=ones_row, start=False, stop=True
                    )
                    nc.scalar.copy(pf_lowT_sb, ps_pl)

                steps = (
                    [lambda c=c: fl_mm(c) for c in range(NCHUNK)]
                    + [fl_done]
                    + [lambda c=c: pl_mm(c) for c in range(NCHUNK)]
                    + [pl_done]
                )
                return (row, r_nat, f_low_sb, h_bias_sb, pf_lowT_sb), steps

            def stage_a(t, lowstate, filler):
                row, r_nat, f_low_sb, h_bias_sb, pf_lowT_sb = lowstate

                def drain(n):
                    for _ in range(n):
                        s = next(filler, None)
                        if s is None:
                            return
                        s()

                # ---- params p1 half (m-major), fused bmm1 ----
                h_dve = small.tile([128, MID], F32)
                prod1 = chunk.tile([128, 32, 128], CDT, tag="prod1", bufs=3)
                for c in range(8):
                    ps_p = psbig.tile([128, 512], F32, tag="big")
                    nc.tensor.matmul(
                        ps_p, lhsT=pf_lowT_sb,
                        rhs=wp_sb[:, c * 512 : (c + 1) * 512],
                        start=True, stop=True,
                    )
                    nc.vector.tensor_mul(
                        prod1[:, c * 4 : (c + 1) * 4, :],
                        ps_p.rearrange("p (m l) -> p m l", l=128),
                        f_low_sb.unsqueeze(1).broadcast_to([128, 4, 128]),
                    )
                    drain(2)
                w = 64
                while w >= 2:
                    nc.vector.tensor_add(
                        prod1[:, :, 0:w], prod1[:, :, 0:w], prod1[:, :, w : 2 * w]
                    )
                    w //= 2
                nc.vector.tensor_add(
                    h_dve.unsqueeze(2), prod1[:, :, 0:1], prod1[:, :, 1:2]
                )

                # ---- h = relu(h_dve + h_bias) ----
                nc.vector.tensor_add(h_dve, h_dve, h_bias_sb)
                h_sb = small.tile([128, MID], CDT)
                nc.scalar.activation(h_sb, h_dve, Relu)

                # ---- params p2 half (l-major), fused bmm2 ----
                g_bf = small.tile([128, LOW], CDT)
                prod2 = chunk.tile([128, 128, MID], CDT, tag="prod2", bufs=3)
                for c in range(8):
                    ps_p = psbig.tile([128, 512], F32, tag="big")
                    nc.tensor.matmul(
                        ps_p, lhsT=pf_lowT_sb,
                        rhs=wp_sb[:, DIM + c * 512 : DIM + (c + 1) * 512],
                        start=True, stop=True,
                    )
                    pchunk = chunk.tile([128, 512], CDT, tag="pchunk", bufs=12)
                    nc.scalar.copy(pchunk, ps_p)
                    nc.gpsimd.tensor_mul(
                        prod2[:, c * 16 : (c + 1) * 16, :],
                        pchunk.rearrange("p (l m) -> p l m", m=32),
                        h_sb.unsqueeze(1).broadcast_to([128, 16, 32]),
                    )
                    drain(1)
                drain(99)
                state = (row, r_nat, h_sb, g_bf, prod2)
                tree2_pending.append(state)
                return state

            def finish_tree2(state):
                # bmm2 tree-reduce, placed where GPSIMD(t) has surely drained
                _row, _r, _h, g_bf, prod2 = state
                w = 16
                while w >= 2:
                    nc.vector.tensor_add(
                        prod2[:, :, 0:w], prod2[:, :, 0:w], prod2[:, :, w : 2 * w]
                    )
                    w //= 2
                nc.vector.tensor_add(
                    g_bf.unsqueeze(2), prod2[:, :, 0:1], prod2[:, :, 1:2]
                )

            def stage_b(state):
                row, r_nat, h_sb, g_bf, prod2 = state
                if tree2_pending and tree2_pending[0] is state:
                    finish_tree2(tree2_pending.pop(0))
                # hT_ext = [h.T ; ones]
                ps_ht = pst.tile([MID, 128], CDT, tag="pst")
                nc.tensor.transpose(ps_ht, h_sb, ident_c)
                hT_ext = small.tile([MID + 1, 128], CDT)
                nc.scalar.copy(hT_ext[:MID, :], ps_ht)
                nc.gpsimd.memset(hT_ext[MID : MID + 1, :], 1.0)

                # gT
                ps_gt = pst.tile([128, 128], CDT, tag="pst")
                nc.tensor.transpose(ps_gt, g_bf, ident_c)
                gT_sb = small.tile([128, 128], CDT)
                nc.scalar.copy(gT_sb, ps_gt)

                # ---- final: out = gT.T @ fin1 + hT_ext.T @ fin2 + r ----
                out_sb = main.tile([128, D], F32)
                for hf in range(2):
                    sl = slice(hf * 512, (hf + 1) * 512)
                    ps_o = psbig.tile([128, 512], F32, tag="big")
                    nc.tensor.matmul(
                        ps_o, lhsT=gT_sb, rhs=fin1_sb[:, sl],
                        start=True, stop=False,
                    )
                    nc.tensor.matmul(
                        ps_o, lhsT=hT_ext, rhs=fin2_sb[:, sl],
                        start=False, stop=True,
                    )
                    nc.vector.tensor_add(out_sb[:, sl], ps_o, r_nat[:, sl])
                nc.scalar.dma_start(out_sh[row, :], out_sb)

            rhs_f_sb = wpool.tile([128, NCHUNK, 160], CDT)
            nc.sync.dma_start(rhs_f_sb, rhs_f.rearrange("(c p) n -> p c n", p=128))
            rhs_pf_sb = wpool.tile([128, NCHUNK, LOW], CDT)
            nc.sync.dma_start(rhs_pf_sb, rhs_pf.rearrange("(c p) n -> p c n", p=128))
            bias_f_sb = wpool.tile([1, 160], CDT)
            nc.sync.dma_start(bias_f_sb, bias_f[:, :])
            bias_pf_sb = wpool.tile([1, LOW], CDT)
            nc.sync.dma_start(bias_pf_sb, bias_pf[:, :])
            loads = [load_tile(0), load_tile(1)]
            wp_sb = wpool.tile([LOW, 2 * DIM], CDT)
            nc.sync.dma_start(wp_sb, wp[:, :])
            fin1_sb = wpool.tile([LOW, D], CDT)
            nc.sync.dma_start(fin1_sb, fin1[:, :])
            fin2_sb = wpool.tile([MID + 1, D], CDT)
            nc.sync.dma_start(fin2_sb, fin2[:, :])


            state0, steps0 = make_low(loads.pop(0))
            for s in steps0:
                s()
            lows = [state0]
            pending = []
            for tp in range(0, NT, 2):
                for t in (tp, tp + 1):
                    if t + 2 < NT:
                        loads.append(load_tile(t + 2))
                    if t + 1 < NT:
                        nstate, nsteps = make_low(loads.pop(0))
                        lows.append(nstate)
                        filler = iter(nsteps)
                    else:
                        filler = iter(())
                    pending.append(stage_a(t, lows.pop(0), filler))
                if len(pending) > 2:
                    stage_b(pending.pop(0))
                    stage_b(pending.pop(0))
            for st in pending:
                stage_b(st)

    _legalize_waits(nc)
    return nc


def _host_prep(proj_f_w, proj_f_b, proj_pf_w, proj_pf_b, proj_f2_w, proj_f2_b,
               pg_w, pg_b):
    B1 = pg_b[:DIM].reshape(LOW, MID)
    B2 = pg_b[DIM:].reshape(MID, LOW)
    W1pT = pg_w[:DIM].reshape(LOW, MID, LOW).transpose(2, 1, 0).reshape(LOW, DIM)
    W2pT = pg_w[DIM:].reshape(MID, LOW, LOW).transpose(2, 1, 0).reshape(LOW, DIM)
    c = np.ascontiguousarray
    return {
        "rhs_f": c(np.concatenate([proj_f_w.T, proj_f_w.T @ B1], axis=1).astype(NP_CDT)),
        "bias_f": c(np.concatenate([proj_f_b, proj_f_b @ B1])[None, :].astype(NP_CDT)),
        "rhs_pf": c(proj_pf_w.T.astype(NP_CDT)),
        "bias_pf": c(proj_pf_b[None, :].astype(NP_CDT)),
        "wp": c(np.concatenate([W1pT, W2pT], axis=1).astype(NP_CDT)),
        "fin1": c(proj_f2_w.T.astype(NP_CDT)),
        "fin2": c(np.concatenate([B2 @ proj_f2_w.T, proj_f2_b[None, :]], axis=0).astype(NP_CDT)),
    }


def kernel(f, pf, proj_f_w, proj_f_b, proj_pf_w, proj_pf_b, proj_f2_w, proj_f2_b,
           pg_w, pg_b):
    f = np.ascontiguousarray(np.asarray(f, dtype=np.float32))
    pf = np.ascontiguousarray(np.asarray(pf, dtype=np.float32))
    weights = _host_prep(
        np.asarray(proj_f_w, np.float32), np.asarray(proj_f_b, np.float32),
        np.asarray(proj_pf_w, np.float32), np.asarray(proj_pf_b, np.float32),
        np.asarray(proj_f2_w, np.float32), np.asarray(proj_f2_b, np.float32),
        np.asarray(pg_w, np.float32), np.asarray(pg_b, np.float32),
    )

    if "nc" not in _CACHED:
        _CACHED["nc"] = _build_nc()
    nc = _CACHED["nc"]

    in_maps = []
    for i in range(N_CORES):
        m = dict(weights)
        m["f_sh"] = f[i * SHARD : (i + 1) * SHARD]
        m["pf_sh"] = pf[i * SHARD : (i + 1) * SHARD]
        in_maps.append(m)

    res = run_bass_kernel_spmd(nc, in_maps, core_ids=list(range(N_CORES)))
    out = np.concatenate([res.results[i]["out_sh"] for i in range(N_CORES)], axis=0)
    return out


# revision 45
# speedup vs baseline: 1.0323x; 1.0013x over previous
"""Trainium2 Bass kernel for nn_DynamicFc (per-sample dynamic MLP).

Strategy: pure data-parallel over 8 NeuronCores (batch 8192 -> 8 x 1024).
Per core, per 128-sample tile:
  f_low   = f @ Wf.T + bf                  (PE, via on-chip transposed f chunks)
  pf_low  = pf @ Wpf.T + bpf               (PE, computed transposed: [low, b])
  params  = pf_low @ pg_w.T (+ pg_b folded via host-side reassociation)
  h = relu(sum_l f_low[b,l] * p1[b,l,m])   (DVE mult+reduce, m-major layout)
  g = sum_m h[b,m] * p2[b,m,l]             (DVE mult+reduce, l-major layout)
  out = g @ W2.T + h @ (B2 @ W2.T) + b2 + f + pf   (PE + DVE/GPSIMD residual)

Bias folding (host-side, exact):
  rhs_f  = [Wf.T | Wf.T @ B1], bias_f = [bf | bf @ B1]  (B1 = pg_b[:4096].reshape(128,32))
  wp     = [W1pT | W2pT]  - pg_w transposed to [low, j] with p1 m-major, p2 l-major
  fin2   = [B2 @ W2.T ; b2]  consumed against [h.T ; ones]
"""

import os
import sys

import numpy as np

for _p in ("/opt/trn_rl_repo", "/root/.axon_site/_ro/trn_rl_repo"):
    if os.path.isdir(_p) and _p not in sys.path:
        sys.path.insert(0, _p)

import bass_rust
import concourse.bass as bass
import concourse.bass_utils as _bu
import concourse.mybir as mybir
import concourse.tile as tile
from concourse.bass_utils import run_bass_kernel_spmd
from concourse.masks import make_identity


def _legalize_waits(nc):
    """walrus (this toolchain) rejects >1 sync wait per instruction; hoist
    extra waits onto preceding NoOps on the same engine."""
    n = 0
    for fn in nc.m.functions:
        for bb in fn.blocks:
            insts = bb.instructions
            out = []
            changed = False
            for inst in insts:
                si = inst.sync_info
                if si is not None and len(si.on_wait) > 1:
                    waits = list(si.on_wait)
                    for w in waits[:-1]:
                        n += 1
                        out.append(
                            mybir.InstNoOp(
                                name=f"I-lw-{n}",
                                engine=inst.engine,
                                sync_info=bass_rust.SyncInfo(on_wait=[w], on_update=[]),
                            )
                        )
                    inst.sync_info = bass_rust.SyncInfo(
                        on_wait=[waits[-1]], on_update=list(si.on_update)
                    )
                    changed = True
                out.append(inst)
            if changed:
                bb.instructions = out
    return n

N_CORES = 8
B, D, LOW, MID = 8192, 1024, 128, 32
DIM = LOW * MID  # 4096
SHARD = B // N_CORES  # 1024
TILE_B = 128
NT = SHARD // TILE_B  # 8
NCHUNK = D // 128  # 8 contraction chunks of the 1024-dim projections

import ml_dtypes

F32 = mybir.dt.float32
CDT = mybir.dt.bfloat16  # compute dtype for matmul operands / prod tiles
NP_CDT = ml_dtypes.bfloat16

_CACHED = {}


def _enable_ldw_opt():
    """Re-enable walrus LDW dedup (safe here: multi-wait legalization already
    keeps instructions single-wait, which was the reason it tripped before)."""
    if _CACHED.get("ldw_patched"):
        return
    orig = _bu.bir_verify_and_optimise

    def patched(tmpdir, inp="bir.json", outp="file.neff", arch=None, *, dve_root=None):
        import pathlib

        cmd = [
            _bu.get_walrus_driver(),
            "--pass",
            "birverifier,runtime_memory_reservation,lower_act,lower_dve,"
            "lower_ap_offset,codegen,neff_packager",
            "-i", inp,
            "--neff-output-filename", outp,
            "--enable-birsim=true", "--mem-mode=physical", "--policy=0",
            "--enable-ldw-opt=true",
            "--assign-static-dmas-to-sp=false",
            "--dram-page-size=256",
            "--enable-neff-debug-info=true",
            "--jobs", "8",
            *_bu.get_walrus_args(
                _bu.get_bir_arch(tmpdir, inp) if arch is None else arch,
                tmpdir, dve_root=dve_root,
            ),
        ]
        r = _bu.run_command(cmd, cwd=tmpdir)
        if r is not None:
            (pathlib.Path(tmpdir) / "log.txt").write_text(r.stdout)
        return f"{tmpdir}/{outp}"

    _bu.bir_verify_and_optimise = patched
    _CACHED["ldw_patched"] = True


def _build_nc():
    nc = bass.Bass()

    f_sh = nc.declare_dram_parameter("f_sh", [SHARD, D], F32, isOutput=False)
    pf_sh = nc.declare_dram_parameter("pf_sh", [SHARD, D], F32, isOutput=False)
    rhs_f = nc.declare_dram_parameter("rhs_f", [D, 160], CDT, isOutput=False)
    bias_f = nc.declare_dram_parameter("bias_f", [1, 160], CDT, isOutput=False)
    rhs_pf = nc.declare_dram_parameter("rhs_pf", [D, LOW], CDT, isOutput=False)
    bias_pf = nc.declare_dram_parameter("bias_pf", [1, LOW], CDT, isOutput=False)
    wp = nc.declare_dram_parameter("wp", [LOW, 2 * DIM], CDT, isOutput=False)
    fin1 = nc.declare_dram_parameter("fin1", [LOW, D], CDT, isOutput=False)
    fin2 = nc.declare_dram_parameter("fin2", [MID + 1, D], CDT, isOutput=False)
    out_sh = nc.declare_dram_parameter("out_sh", [SHARD, D], F32, isOutput=True)

    Relu = mybir.ActivationFunctionType.Relu

    with tile.TileContext(nc) as tc:
        with (
            tc.tile_pool(name="wpool", bufs=1) as wpool,
            tc.tile_pool(name="main", bufs=4) as main,
            tc.tile_pool(name="small", bufs=3) as small,
            tc.tile_pool(name="chunk", bufs=3) as chunk,
            tc.tile_pool(name="pst", bufs=1, space="PSUM") as pst,
            tc.tile_pool(name="pslow", bufs=2, space="PSUM") as pslow,
            tc.tile_pool(name="psbig", bufs=5, space="PSUM") as psbig,
        ):
            # ---- one-time constants / weights ----
            ident = wpool.tile([128, 128], F32)
            make_identity(nc, ident)
            ident_c = wpool.tile([128, 128], CDT)
            make_identity(nc, ident_c)
            ones_row = wpool.tile([1, 128], CDT)
            nc.gpsimd.memset(ones_row, 1.0)

            _loaded0 = None
            tree2_pending = []

            def load_tile(t):
                row = slice(t * TILE_B, (t + 1) * TILE_B)
                r_nat = main.tile([128, D], F32)
                nc.scalar.dma_start(r_nat, r_sh[row, :])
                # transposed loads straight from DRAM via the DMA xbar
                fT = main.tile([128, NCHUNK, 128], CDT)
                nc.sync.dma_start_transpose(fT, f16_sh[row, :])
                pfT = main.tile([128, NCHUNK, 128], CDT)
                nc.sync.dma_start_transpose(pfT, pf16_sh[row, :])
                return row, r_nat, fT, pfT

            def make_low(loaded):
                # low-phase matmuls as step closures, interleaved into the
                # previous tile's params phase as in-order-PE filler work
                row, r_nat, fT, pfT = loaded
                ps_fl = pslow.tile([128, 160], F32, tag="lowp")
                ps_pl = pslow.tile([128, 128], F32, tag="lowp")
                f_low_sb = small.tile([128, LOW], CDT)
                h_bias_sb = small.tile([128, MID], F32)
                pf_lowT_sb = small.tile([128, 128], CDT)

                def fl_mm(c):
                    nc.tensor.matmul(
                        ps_fl, lhsT=fT[:, c, :], rhs=rhs_f_sb[:, c, :],
                        start=(c == 0), stop=False,
                    )

                def fl_done():
                    nc.tensor.matmul(
                        ps_fl, lhsT=ones_row, rhs=bias_f_sb, start=False, stop=True
                    )
                    nc.scalar.copy(f_low_sb, ps_fl[:, :LOW])
                    nc.scalar.copy(h_bias_sb, ps_fl[:, LOW:])

                def pl_mm(c):
                    nc.tensor.matmul(
                        ps_pl, lhsT=rhs_pf_sb[:, c, :], rhs=pfT[:, c, :],
                        start=(c == 0), stop=False,
                    )

                def pl_done():
                    nc.tensor.matmul(
                        ps_pl, lhsT=bias_pf_sb, rhs=ones_row, start=False, stop=True
                    )
                    nc.scalar.copy(pf_lowT_sb, ps_pl)

                steps = (
                    [lambda c=c: fl_mm(c) for c in range(NCHUNK)]
                    + [fl_done]
                    + [lambda c=c: pl_mm(c) for c in range(NCHUNK)]
                    + [pl_done]
                )
                return (row, r_nat, f_low_sb, h_bias_sb, pf_lowT_sb), steps

            def stage_a(t, lowstate, filler):
                row, r_nat, f_low_sb, h_bias_sb, pf_lowT_sb = lowstate

                def drain(n):
                    for _ in range(n):
                        s = next(filler, None)
                        if s is None:
                            return
                        s()

                # ---- params p1 half (m-major), fused bmm1 ----
                h_dve = small.tile([128, MID], F32)
                prod1 = chunk.tile([128, 32, 128], CDT, tag="prod1", bufs=3)
                for c in range(8):
                    ps_p = psbig.tile([128, 512], F32, tag="big")
                    nc.tensor.matmul(
                        ps_p, lhsT=pf_lowT_sb,
                        rhs=wp_sb[:, c * 512 : (c + 1) * 512],
                        start=True, stop=True,
                    )
                    nc.vector.tensor_mul(
                        prod1[:, c * 4 : (c + 1) * 4, :],
                        ps_p.rearrange("p (m l) -> p m l", l=128),
                        f_low_sb.unsqueeze(1).broadcast_to([128, 4, 128]),
                    )
                    drain(3)
                w = 64
                while w >= 2:
                    nc.vector.tensor_add(
                        prod1[:, :, 0:w], prod1[:, :, 0:w], prod1[:, :, w : 2 * w]
                    )
                    w //= 2
                nc.vector.tensor_add(
                    h_dve.unsqueeze(2), prod1[:, :, 0:1], prod1[:, :, 1:2]
                )

                # ---- h = relu(h_dve + h_bias) ----
                nc.vector.tensor_add(h_dve, h_dve, h_bias_sb)
                h_sb = small.tile([128, MID], CDT)
                nc.scalar.activation(h_sb, h_dve, Relu)

                # ---- params p2 half (l-major), fused bmm2 ----
                g_bf = small.tile([128, LOW], CDT)
                prod2 = chunk.tile([128, 128, MID], CDT, tag="prod2", bufs=3)
                for c in range(8):
                    ps_p = psbig.tile([128, 512], F32, tag="big")
                    nc.tensor.matmul(
                        ps_p, lhsT=pf_lowT_sb,
                        rhs=wp_sb[:, DIM + c * 512 : DIM + (c + 1) * 512],
                        start=True, stop=True,
                    )
                    pchunk = chunk.tile([128, 512], CDT, tag="pchunk", bufs=12)
                    nc.scalar.copy(pchunk, ps_p)
                    nc.gpsimd.tensor_mul(
                        prod2[:, c * 16 : (c + 1) * 16, :],
                        pchunk.rearrange("p (l m) -> p l m", m=32),
                        h_sb.unsqueeze(1).broadcast_to([128, 16, 32]),
                    )
                drain(99)
                state = (row, r_nat, h_sb, g_bf, prod2)
                tree2_pending.append(state)
                return state

            def finish_tree2(state):
                # bmm2 tree-reduce, placed where GPSIMD(t) has surely drained
                _row, _r, _h, g_bf, prod2 = state
                w = 16
                while w >= 2:
                    nc.vector.tensor_add(
                        prod2[:, :, 0:w], prod2[:, :, 0:w], prod2[:, :, w : 2 * w]
                    )
                    w //= 2
                nc.vector.tensor_add(
                    g_bf.unsqueeze(2), prod2[:, :, 0:1], prod2[:, :, 1:2]
                )

            def stage_b(state):
                row, r_nat, h_sb, g_bf, prod2 = state
                if tree2_pending and tree2_pending[0] is state:
                    finish_tree2(tree2_pending.pop(0))
                # hT_ext = [h.T ; ones]
                ps_ht = pst.tile([MID, 128], CDT, tag="pst")
                nc.tensor.transpose(ps_ht, h_sb, ident_c)
                hT_ext = small.tile([MID + 1, 128], CDT)
                nc.scalar.copy(hT_ext[:MID, :], ps_ht)
                nc.gpsimd.memset(hT_ext[MID : MID + 1, :], 1.0)

                # gT
                ps_gt = pst.tile([128, 128], CDT, tag="pst")
                nc.tensor.transpose(ps_gt, g_bf, ident_c)
                gT_sb = small.tile([128, 128], CDT)
                nc.scalar.copy(gT_sb, ps_gt)

                # ---- final: out = gT.T @ fin1 + hT_ext.T @ fin2 + r ----
                out_sb = main.tile([128, D], F32)
                for hf in range(2):
                    sl = slice(hf * 512, (hf + 1) * 512)
                    ps_o = psbig.tile([128, 512], F32, tag="big")
                    nc.tensor.matmul(
                        ps_o, lhsT=gT_sb, rhs=fin1_sb[:, sl],
                        start=True, stop=False,
                    )
                    nc.tensor.matmul(
                        ps_o, lhsT=hT_ext, rhs=fin2_sb[:, sl],
                        start=False, stop=True,
                    )
                    nc.vector.tensor_add(out_sb[:, sl], ps_o, r_nat[:, sl])
                nc.scalar.dma_start(out_sh[row, :], out_sb)

            rhs_f_sb = wpool.tile([128, NCHUNK, 160], CDT)
            nc.sync.dma_start(rhs_f_sb, rhs_f.rearrange("(c p) n -> p c n", p=128))
            rhs_pf_sb = wpool.tile([128, NCHUNK, LOW], CDT)
            nc.sync.dma_start(rhs_pf_sb, rhs_pf.rearrange("(c p) n -> p c n", p=128))
            bias_f_sb = wpool.tile([1, 160], CDT)
            nc.sync.dma_start(bias_f_sb, bias_f[:, :])
            bias_pf_sb = wpool.tile([1, LOW], CDT)
            nc.sync.dma_start(bias_pf_sb, bias_pf[:, :])
            loads = [load_tile(0), load_tile(1)]
            wp_sb = wpool.tile([LOW, 2 * DIM], CDT)
            nc.sync.dma_start(wp_sb, wp[:, :])
            fin1_sb = wpool.tile([LOW, D], CDT)
            nc.sync.dma_start(fin1_sb, fin1[:, :])
            fin2_sb = wpool.tile([MID + 1, D], CDT)
            nc.sync.dma_start(fin2_sb, fin2[:, :])


            state0, steps0 = make_low(loads.pop(0))
            for s in steps0:
                s()
            lows = [state0]
            pending = []
            for tp in range(0, NT, 2):
                for t in (tp, tp + 1):
                    if t + 2 < NT:
                        loads.append(load_tile(t + 2))
                    if t + 1 < NT:
                        nstate, nsteps = make_low(loads.pop(0))
                        lows.append(nstate)
                        filler = iter(nsteps)
                    else:
                        filler = iter(())
                    pending.append(stage_a(t, lows.pop(0), filler))
                if len(pending) > 2:
                    stage_b(pending.pop(0))
                    stage_b(pending.pop(0))
            for st in pending:
                stage_b(st)

    _legalize_waits(nc)
    return nc


def _host_prep(proj_f_w, proj_f_b, proj_pf_w, proj_pf_b, proj_f2_w, proj_f2_b,
               pg_w, pg_b):
    B1 = pg_b[:DIM].reshape(LOW, MID)
    B2 = pg_b[DIM:].reshape(MID, LOW)
    W1pT = pg_w[:DIM].reshape(LOW, MID, LOW).transpose(2, 1, 0).reshape(LOW, DIM)
    W2pT = pg_w[DIM:].reshape(MID, LOW, LOW).transpose(2, 1, 0).reshape(LOW, DIM)
    c = np.ascontiguousarray
    return {
        "rhs_f": c(np.concatenate([proj_f_w.T, proj_f_w.T @ B1], axis=1).astype(NP_CDT)),
        "bias_f": c(np.concatenate([proj_f_b, proj_f_b @ B1])[None, :].astype(NP_CDT)),
        "rhs_pf": c(proj_pf_w.T.astype(NP_CDT)),
        "bias_pf": c(proj_pf_b[None, :].astype(NP_CDT)),
        "wp": c(np.concatenate([W1pT, W2pT], axis=1).astype(NP_CDT)),
        "fin1": c(proj_f2_w.T.astype(NP_CDT)),
        "fin2": c(np.concatenate([B2 @ proj_f2_w.T, proj_f2_b[None, :]], axis=0).astype(NP_CDT)),
    }


def kernel(f, pf, proj_f_w, proj_f_b, proj_pf_w, proj_pf_b, proj_f2_w, proj_f2_b,
           pg_w, pg_b):
    f = np.ascontiguousarray(np.asarray(f, dtype=np.float32))
    pf = np.ascontiguousarray(np.asarray(pf, dtype=np.float32))
    weights = _host_prep(
        np.asarray(proj_f_w, np.float32), np.asarray(proj_f_b, np.float32),
        np.asarray(proj_pf_w, np.float32), np.asarray(proj_pf_b, np.float32),
        np.asarray(proj_f2_w, np.float32), np.asarray(proj_f2_b, np.float32),
        np.asarray(pg_w, np.float32), np.asarray(pg_b, np.float32),
    )

    if "nc" not in _CACHED:
        _CACHED["nc"] = _build_nc()
    nc = _CACHED["nc"]

    in_maps = []
    for i in range(N_CORES):
        m = dict(weights)
        m["f_sh"] = f[i * SHARD : (i + 1) * SHARD]
        m["pf_sh"] = pf[i * SHARD : (i + 1) * SHARD]
        in_maps.append(m)

    res = run_bass_kernel_spmd(nc, in_maps, core_ids=list(range(N_CORES)))
    out = np.concatenate([res.results[i]["out_sh"] for i in range(N_CORES)], axis=0)
    return out
